# revision 6
# baseline (speedup 1.0000x reference)
"""Trainium2 kernel for 4096x4096 single-channel 7x7 valid cross-correlation + bias.

Strategy (v7)
-------------
Conv decomposed into 7 banded-Toeplitz matmuls accumulated in PSUM:

    y[r, c] = sum_j sum_i W[i, j] * x[r+i, c+j]

Per strip of 128 input rows ([K=128 partitions, width] SBUF tile), kernel
column j contributes one TensorEngine matmul:
    lhsT = T_j [128, 128] with T_j[k, m] = W[k-m, j]   (stationary, banded)
    rhs  = X[:, j : j+512]                              (free-dim shift)
accumulating 122 valid output rows x 512 output cols in one PSUM bank.

Sharding: columns across 8 cores (512 output cols each + 6-col halo sliced
host-side).  34 row strips per core, two strips packed per 128-partition
SBUF line ("pair").

Schedule (evolved v2->v7 via neuron-profile traces):
- PSUM groups of 4 strips (last group 6) with bank phasing: group g and g+1
  use disjoint PSUM bank sets, so the PE never waits on drains.
- Drains alternate ScalarE (even strip) / VectorE (odd strip).
- All DMA on the sync-engine hardware DGE ring (16 SDMA engines). Stores are
  padded to 128 partitions: the HWDGE only splits a DMA across all 16 SDMA
  engines when the SBUF-side partition count is divisible by 8 (2-engine
  ~49 GB/s fallback otherwise -- probe-verified).
- DMA instruction count minimized (9 loads, 5 stores): every DMA costs a
  semaphore, and the TileContext epilogue clears semaphores one instruction
  apiece on the issuing engine (~115 ns each) inside the measured span.
  Loads are chunked with fine granularity early (pair 0 split in half) and
  coarse late; stores are chunked 4 pairs apiece (last chunk = 1 pair so the
  post-compute flush is small).
- DRAM layouts are pair-inner ([128, N_PAIRS, w]) so chunked transfers
  balance into 3-dim access patterns with 2 KB descriptors.
"""

import os

import numpy as np
import ml_dtypes

import concourse.bass as bass
import concourse.bacc as bacc_mod
import concourse.mybir as mybir
import concourse.tile as tile
from concourse.bass_utils import run_bass_kernel_spmd

H = 4096          # input rows
W = 4096          # input cols
KH = 7            # kernel rows
KW = 7            # kernel cols
OH = H - KH + 1   # 4090 output rows
OW = W - KW + 1   # 4090 output cols
NCORES = 8
CW = 512          # output cols per core
SW = CW + KW - 1  # 518 input cols per shard
STRIP = 122       # output rows per strip (128 input rows -> 122 outputs)
N_STRIPS = -(-OH // STRIP)   # 34
N_PAIRS = -(-N_STRIPS // 2)  # 17
GROUP = 4                    # strips per j-outer group (PSUM bank phasing)

# input-load chunks (pair ranges); fine-grained early, coarse late
LOAD_CHUNKS = [(2, 4), (4, 7), (7, 12), (12, 17)]
# output-store chunks (pair ranges); last chunk small for a short tail
STORE_CHUNKS = [(0, 4), (4, 8), (8, 12), (12, 16), (16, 17)]

_BF16 = ml_dtypes.bfloat16


def _strip_mk(s: int) -> tuple[int, int]:
    """(valid output rows, input rows) of strip s."""
    m = min(STRIP, OH - s * STRIP)
    return m, m + KH - 1


def _build_program(bias_val: float) -> bass.Bass:
    nc = bacc_mod.Bacc("TRN2", target_bir_lowering=False)

    x_d = nc.dram_tensor("xs", [128, N_PAIRS, 2 * SW], mybir.dt.bfloat16,
                         kind="ExternalInput")
    t_d = nc.dram_tensor("tmat", [128, KW * 128], mybir.dt.bfloat16,
                         kind="ExternalInput")
    y_d = nc.dram_tensor("y", [128, N_PAIRS, 2 * CW], mybir.dt.bfloat16,
                         kind="ExternalOutput")

    with tile.TileContext(nc) as tc:
        with (
            tc.tile_pool(name="const", bufs=1) as constp,
            tc.tile_pool(name="xg", bufs=2 + len(LOAD_CHUNKS)) as xgp,
            tc.tile_pool(name="yg", bufs=len(STORE_CHUNKS)) as ygp,
            tc.tile_pool(name="ps", bufs=8, space="PSUM") as psp,
        ):
            t_sb = constp.tile([128, KW * 128], mybir.dt.bfloat16)

            # pair -> (sbuf tile, element offset of the pair inside it)
            xg_at = {}
            p0 = xgp.tile([128, 2 * SW], mybir.dt.bfloat16, name="p0", tag="xg")
            p1 = xgp.tile([128, 2 * SW], mybir.dt.bfloat16, name="p1", tag="xg")
            xg_at[0] = (p0, 0)
            xg_at[1] = (p1, 0)

            nc.sync.dma_start(t_sb[:, :256], t_d[:, :256])
            nc.sync.dma_start(p0[:, :SW], x_d[:, 0, :SW])
            nc.sync.dma_start(p0[:, SW:], x_d[:, 0, SW:])
            nc.sync.dma_start(p1[:, :], x_d[:, 1, :])
            nc.sync.dma_start(t_sb[:, 256:], t_d[:, 256:])
            for g0, g1 in LOAD_CHUNKS:
                ch = xgp.tile([128, (g1 - g0) * 2 * SW], mybir.dt.bfloat16,
                              name=f"x{g0}", tag="xg")
                for g in range(g0, g1):
                    xg_at[g] = (ch, (g - g0) * 2 * SW)
                nc.sync.dma_start(ch[:, :], x_d[:, g0:g1, :])

            # pair -> (store-chunk tile, element offset); created lazily
            yg_at = {}
            store_of = {g0: (g0, g1) for g0, g1 in STORE_CHUNKS}

            bounds = list(range(0, N_STRIPS - 6, GROUP)) + [N_STRIPS - 6]
            for gi, b0 in enumerate(bounds):
                b1 = bounds[gi + 1] if gi + 1 < len(bounds) else N_STRIPS
                strips = list(range(b0, b1))

                ps_tiles = {}
                for s in strips:
                    ps_tiles[s] = psp.tile([128, CW], mybir.dt.float32,
                                           name="ps", tag="ps")

                for j in range(KW):
                    for s in strips:
                        m, k = _strip_mk(s)
                        mw = 128 if m == STRIP else m
                        xg, xoff = xg_at[s // 2]
                        off = xoff + (s % 2) * SW
                        nc.tensor.matmul(
                            ps_tiles[s][:mw, :],
                            t_sb[:k, j * 128:j * 128 + mw],
                            xg[:k, off + j:off + j + CW],
                            start=(j == 0),
                            stop=(j == KW - 1),
                        )

                for s in strips:
                    m, _ = _strip_mk(s)
                    g, h = s // 2, s % 2
                    if g in store_of and h == 0 and g not in yg_at:
                        c0, c1 = store_of[g]
                        yg = ygp.tile([128, (c1 - c0) * 2 * CW],
                                      mybir.dt.bfloat16, name=f"y{c0}", tag="yg")
                        for gg in range(c0, c1):
                            yg_at[gg] = (yg, (gg - c0) * 2 * CW)
                    yg, yoff = yg_at[g]
                    dst = yg[:m, yoff + h * CW:yoff + (h + 1) * CW]
                    src = ps_tiles[s][:m, :]
                    if s % 2 == 0:
                        nc.scalar.activation(
                            dst, src, mybir.ActivationFunctionType.Copy,
                            bias=float(bias_val),
                        )
                    else:
                        nc.vector.tensor_scalar_add(dst, src, float(bias_val))

                    # fire the chunk store after its last strip's drain
                    for c0, c1 in STORE_CHUNKS:
                        last_strip = min(2 * c1 - 1, N_STRIPS - 1)
                        if s == last_strip:
                            ygc, _ = yg_at[c0]
                            nc.sync.dma_start(y_d[:, c0:c1, :], ygc[:, :])

    nc.compile()
    nc.finalize()
    return nc


def _toeplitz(weight: np.ndarray) -> np.ndarray:
    """[128, 7*128] bf16; block j holds T_j[k, m] = W[k-m, j] (band 0<=k-m<7)."""
    t = np.zeros((128, KW * 128), np.float32)
    for j in range(KW):
        for i in range(KH):
            mm = np.arange(0, 128 - i)
            t[mm + i, j * 128 + mm] = weight[i, j]
    return t.astype(_BF16)


def _pack_shard(x_bf: np.ndarray, c0: int) -> np.ndarray:
    """[128, 17, 2*518] bf16 (pair-inner): two strips per partition line."""
    valid = min(SW, W - c0)
    xs = np.zeros((H + 2 * STRIP, SW), _BF16)  # row padding for edge strips
    xs[:H, :valid] = x_bf[:, c0:c0 + valid]
    packed = np.zeros((N_PAIRS, 128, 2 * SW), _BF16)
    for g in range(N_PAIRS):
        packed[g, :, :SW] = xs[2 * g * STRIP: 2 * g * STRIP + 128]
        packed[g, :, SW:] = xs[(2 * g + 1) * STRIP: (2 * g + 1) * STRIP + 128]
    return np.ascontiguousarray(packed.transpose(1, 0, 2))


def _unpack_out(y_packed: np.ndarray) -> np.ndarray:
    """[128, 17, 1024] bf16 (pair-inner, 128-row padded) -> [4090, 512] f32."""
    out = np.empty((OH, CW), np.float32)
    for s in range(N_STRIPS):
        m, _ = _strip_mk(s)
        g, h = s // 2, s % 2
        out[s * STRIP: s * STRIP + m, :] = \
            y_packed[:m, g, h * CW:(h + 1) * CW].astype(np.float32)
    return out


def kernel(x: np.ndarray, weight: np.ndarray, bias: np.ndarray) -> np.ndarray:
    x = np.asarray(x, dtype=np.float32)
    weight = np.asarray(weight, dtype=np.float32)
    bias = np.asarray(bias, dtype=np.float32)

    tmat = _toeplitz(weight)
    x_bf = x.astype(_BF16)

    in_maps = []
    for c in range(NCORES):
        in_maps.append({"xs": _pack_shard(x_bf, CW * c), "tmat": tmat})

    nc = _build_program(float(bias[0]))

    trace = bool(int(os.environ.get("CONV_KERNEL_TRACE", "0")))
    res = run_bass_kernel_spmd(nc, in_maps, core_ids=list(range(NCORES)),
                               trace=trace)
    if trace:
        kernel.last_exec_time_ns = res.exec_time_ns

    cols = []
    for c in range(NCORES):
        valid_out = min(CW, OW - CW * c)
        cols.append(_unpack_out(np.asarray(res.results[c]["y"]))[:, :valid_out])
    return np.concatenate(cols, axis=1).astype(np.float32)


# revision 7
# speedup vs baseline: 1.0135x; 1.0135x over previous
"""Trainium2 kernel for 4096x4096 single-channel 7x7 valid cross-correlation + bias.

Strategy (v7)
-------------
Conv decomposed into 7 banded-Toeplitz matmuls accumulated in PSUM:

    y[r, c] = sum_j sum_i W[i, j] * x[r+i, c+j]

Per strip of 128 input rows ([K=128 partitions, width] SBUF tile), kernel
column j contributes one TensorEngine matmul:
    lhsT = T_j [128, 128] with T_j[k, m] = W[k-m, j]   (stationary, banded)
    rhs  = X[:, j : j+512]                              (free-dim shift)
accumulating 122 valid output rows x 512 output cols in one PSUM bank.

Sharding: columns across 8 cores (512 output cols each + 6-col halo sliced
host-side).  34 row strips per core, two strips packed per 128-partition
SBUF line ("pair").

Schedule (evolved v2->v7 via neuron-profile traces):
- PSUM groups of 4 strips (last group 6) with bank phasing: group g and g+1
  use disjoint PSUM bank sets, so the PE never waits on drains.
- Drains alternate ScalarE (even strip) / VectorE (odd strip).
- All DMA on the sync-engine hardware DGE ring (16 SDMA engines). Stores are
  padded to 128 partitions: the HWDGE only splits a DMA across all 16 SDMA
  engines when the SBUF-side partition count is divisible by 8 (2-engine
  ~49 GB/s fallback otherwise -- probe-verified).
- DMA instruction count minimized (9 loads, 5 stores): every DMA costs a
  semaphore, and the TileContext epilogue clears semaphores one instruction
  apiece on the issuing engine (~115 ns each) inside the measured span.
  Loads are chunked with fine granularity early (pair 0 split in half) and
  coarse late; stores are chunked 4 pairs apiece (last chunk = 1 pair so the
  post-compute flush is small).
- DRAM layouts are pair-inner ([128, N_PAIRS, w]) so chunked transfers
  balance into 3-dim access patterns with 2 KB descriptors.
"""

import os

import numpy as np
import ml_dtypes

import concourse.bass as bass
import concourse.bacc as bacc_mod
import concourse.mybir as mybir
import concourse.tile as tile
from concourse.bass_utils import run_bass_kernel_spmd

H = 4096          # input rows
W = 4096          # input cols
KH = 7            # kernel rows
KW = 7            # kernel cols
OH = H - KH + 1   # 4090 output rows
OW = W - KW + 1   # 4090 output cols
NCORES = 8
CW = 512          # output cols per core
SW = CW + KW - 1  # 518 input cols per shard
STRIP = 122       # output rows per strip (128 input rows -> 122 outputs)
N_STRIPS = -(-OH // STRIP)   # 34
N_PAIRS = -(-N_STRIPS // 2)  # 17
GROUP = 4                    # strips per j-outer group (PSUM bank phasing)

# input-load chunks (pair ranges); fine-grained early, coarse late
LOAD_CHUNKS = [(2, 4), (4, 7), (7, 12), (12, 17)]
# output-store chunks (pair ranges); last chunk small for a short tail
STORE_CHUNKS = [(0, 5), (5, 10), (10, 13), (13, 15), (15, 16), (16, 17)]

_BF16 = ml_dtypes.bfloat16


def _strip_mk(s: int) -> tuple[int, int]:
    """(valid output rows, input rows) of strip s."""
    m = min(STRIP, OH - s * STRIP)
    return m, m + KH - 1


def _build_program(bias_val: float) -> bass.Bass:
    nc = bacc_mod.Bacc("TRN2", target_bir_lowering=False)

    x_d = nc.dram_tensor("xs", [128, N_PAIRS, 2 * SW], mybir.dt.bfloat16,
                         kind="ExternalInput")
    t_d = nc.dram_tensor("tmat", [128, KW * 128], mybir.dt.bfloat16,
                         kind="ExternalInput")
    y_d = nc.dram_tensor("y", [128, N_PAIRS, 2 * CW], mybir.dt.bfloat16,
                         kind="ExternalOutput")

    with tile.TileContext(nc) as tc:
        with (
            tc.tile_pool(name="const", bufs=1) as constp,
            tc.tile_pool(name="xg", bufs=2 + len(LOAD_CHUNKS)) as xgp,
            tc.tile_pool(name="yg", bufs=len(STORE_CHUNKS)) as ygp,
            tc.tile_pool(name="ps", bufs=8, space="PSUM") as psp,
        ):
            t_sb = constp.tile([128, KW * 128], mybir.dt.bfloat16)

            # pair -> (sbuf tile, element offset of the pair inside it)
            xg_at = {}
            p0 = xgp.tile([128, 2 * SW], mybir.dt.bfloat16, name="p0", tag="xg")
            p1 = xgp.tile([128, 2 * SW], mybir.dt.bfloat16, name="p1", tag="xg")
            xg_at[0] = (p0, 0)
            xg_at[1] = (p1, 0)

            nc.sync.dma_start(t_sb[:, :256], t_d[:, :256])
            nc.sync.dma_start(p0[:, :SW], x_d[:, 0, :SW])
            nc.sync.dma_start(p0[:, SW:], x_d[:, 0, SW:])
            nc.sync.dma_start(p1[:, :], x_d[:, 1, :])
            nc.sync.dma_start(t_sb[:, 256:], t_d[:, 256:])
            for g0, g1 in LOAD_CHUNKS:
                ch = xgp.tile([128, (g1 - g0) * 2 * SW], mybir.dt.bfloat16,
                              name=f"x{g0}", tag="xg")
                for g in range(g0, g1):
                    xg_at[g] = (ch, (g - g0) * 2 * SW)
                nc.sync.dma_start(ch[:, :], x_d[:, g0:g1, :])

            # pair -> (store-chunk tile, element offset); created lazily
            yg_at = {}
            store_of = {g0: (g0, g1) for g0, g1 in STORE_CHUNKS}

            bounds = list(range(0, N_STRIPS - 6, GROUP)) + [N_STRIPS - 6]
            for gi, b0 in enumerate(bounds):
                b1 = bounds[gi + 1] if gi + 1 < len(bounds) else N_STRIPS
                strips = list(range(b0, b1))

                ps_tiles = {}
                for s in strips:
                    ps_tiles[s] = psp.tile([128, CW], mybir.dt.float32,
                                           name="ps", tag="ps")

                for j in range(KW):
                    for s in strips:
                        m, k = _strip_mk(s)
                        mw = 128 if m == STRIP else m
                        xg, xoff = xg_at[s // 2]
                        off = xoff + (s % 2) * SW
                        nc.tensor.matmul(
                            ps_tiles[s][:mw, :],
                            t_sb[:k, j * 128:j * 128 + mw],
                            xg[:k, off + j:off + j + CW],
                            start=(j == 0),
                            stop=(j == KW - 1),
                        )

                for s in strips:
                    m, _ = _strip_mk(s)
                    g, h = s // 2, s % 2
                    if g in store_of and h == 0 and g not in yg_at:
                        c0, c1 = store_of[g]
                        yg = ygp.tile([128, (c1 - c0) * 2 * CW],
                                      mybir.dt.bfloat16, name=f"y{c0}", tag="yg")
                        for gg in range(c0, c1):
                            yg_at[gg] = (yg, (gg - c0) * 2 * CW)
                    yg, yoff = yg_at[g]
                    dst = yg[:m, yoff + h * CW:yoff + (h + 1) * CW]
                    src = ps_tiles[s][:m, :]
                    if s % 2 == 0:
                        nc.scalar.activation(
                            dst, src, mybir.ActivationFunctionType.Copy,
                            bias=float(bias_val),
                        )
                    else:
                        nc.vector.tensor_scalar_add(dst, src, float(bias_val))

                    # fire the chunk store after its last strip's drain
                    for c0, c1 in STORE_CHUNKS:
                        last_strip = min(2 * c1 - 1, N_STRIPS - 1)
                        if s == last_strip:
                            ygc, _ = yg_at[c0]
                            nc.sync.dma_start(y_d[:, c0:c1, :], ygc[:, :])

    nc.compile()
    nc.finalize()
    return nc


def _toeplitz(weight: np.ndarray) -> np.ndarray:
    """[128, 7*128] bf16; block j holds T_j[k, m] = W[k-m, j] (band 0<=k-m<7)."""
    t = np.zeros((128, KW * 128), np.float32)
    for j in range(KW):
        for i in range(KH):
            mm = np.arange(0, 128 - i)
            t[mm + i, j * 128 + mm] = weight[i, j]
    return t.astype(_BF16)


def _pack_shard(x_bf: np.ndarray, c0: int) -> np.ndarray:
    """[128, 17, 2*518] bf16 (pair-inner): two strips per partition line."""
    valid = min(SW, W - c0)
    xs = np.zeros((H + 2 * STRIP, SW), _BF16)  # row padding for edge strips
    xs[:H, :valid] = x_bf[:, c0:c0 + valid]
    packed = np.zeros((N_PAIRS, 128, 2 * SW), _BF16)
    for g in range(N_PAIRS):
        packed[g, :, :SW] = xs[2 * g * STRIP: 2 * g * STRIP + 128]
        packed[g, :, SW:] = xs[(2 * g + 1) * STRIP: (2 * g + 1) * STRIP + 128]
    return np.ascontiguousarray(packed.transpose(1, 0, 2))


def _unpack_out(y_packed: np.ndarray) -> np.ndarray:
    """[128, 17, 1024] bf16 (pair-inner, 128-row padded) -> [4090, 512] f32."""
    out = np.empty((OH, CW), np.float32)
    for s in range(N_STRIPS):
        m, _ = _strip_mk(s)
        g, h = s // 2, s % 2
        out[s * STRIP: s * STRIP + m, :] = \
            y_packed[:m, g, h * CW:(h + 1) * CW].astype(np.float32)
    return out


def kernel(x: np.ndarray, weight: np.ndarray, bias: np.ndarray) -> np.ndarray:
    x = np.asarray(x, dtype=np.float32)
    weight = np.asarray(weight, dtype=np.float32)
    bias = np.asarray(bias, dtype=np.float32)

    tmat = _toeplitz(weight)
    x_bf = x.astype(_BF16)

    in_maps = []
    for c in range(NCORES):
        in_maps.append({"xs": _pack_shard(x_bf, CW * c), "tmat": tmat})

    nc = _build_program(float(bias[0]))

    trace = bool(int(os.environ.get("CONV_KERNEL_TRACE", "0")))
    res = run_bass_kernel_spmd(nc, in_maps, core_ids=list(range(NCORES)),
                               trace=trace)
    if trace:
        kernel.last_exec_time_ns = res.exec_time_ns

    cols = []
    for c in range(NCORES):
        valid_out = min(CW, OW - CW * c)
        cols.append(_unpack_out(np.asarray(res.results[c]["y"]))[:, :valid_out])
    return np.concatenate(cols, axis=1).astype(np.float32)


# revision 8
# speedup vs baseline: 1.0197x; 1.0061x over previous
"""Trainium2 kernel for 4096x4096 single-channel 7x7 valid cross-correlation + bias.

Strategy (v9)
-------------
Conv decomposed into 7 banded-Toeplitz matmuls accumulated in PSUM:

    y[r, c] = sum_j sum_i W[i, j] * x[r+i, c+j]

Per strip of 128 input rows ([K=128 partitions, width] SBUF tile), kernel
column j contributes one TensorEngine matmul:
    lhsT = T_j [128, 128] with T_j[k, m] = W[k-m, j]   (stationary, banded)
    rhs  = X[:, j : j+512]                              (free-dim shift)
accumulating 122 valid output rows x 512 output cols in one PSUM bank.

Sharding: columns across 8 cores (512 output cols each + 6-col halo sliced
host-side).  34 row strips per core, two strips packed per 128-partition
SBUF line ("pair").

Schedule (evolved v2->v9 via neuron-profile traces):
- PSUM groups of 4 strips (last group 6) with bank phasing: group g and g+1
  use disjoint PSUM bank sets, so the PE never waits on drains.
- j order zigzags across groups (even groups 0..6, odd groups 6..0) so the
  weights loaded in the PE array at a group boundary are reused by the next
  group's first matmul.
- Drains alternate ScalarE (even strip) / VectorE (odd strip).
- All DMA on the sync-engine hardware DGE ring (16 SDMA engines). Stores are
  padded to 128 partitions: the HWDGE only splits a DMA across all 16 SDMA
  engines when the SBUF-side partition count is divisible by 8 (2-engine
  ~49 GB/s fallback otherwise -- probe-verified). Per-pair stores fire as
  soon as both strips of the pair are drained.
- Load order: T_0/T_1 block, pair-0 halves, pair 1, rest of tmat, then the
  remaining pairs -- the first matmul starts ~1.5 us earlier than with
  monolithic loads.
- The TileContext/runtime epilogue (full 256-semaphore clear sweep, ~8 us)
  is a fixed cost independent of program structure; don't chase it.
"""

import os

import numpy as np
import ml_dtypes

import concourse.bass as bass
import concourse.bacc as bacc_mod
import concourse.mybir as mybir
import concourse.tile as tile
from concourse.bass_utils import run_bass_kernel_spmd

H = 4096          # input rows
W = 4096          # input cols
KH = 7            # kernel rows
KW = 7            # kernel cols
OH = H - KH + 1   # 4090 output rows
OW = W - KW + 1   # 4090 output cols
NCORES = 8
CW = 512          # output cols per core
SW = CW + KW - 1  # 518 input cols per shard
STRIP = 122       # output rows per strip (128 input rows -> 122 outputs)
N_STRIPS = -(-OH // STRIP)   # 34
N_PAIRS = -(-N_STRIPS // 2)  # 17
GROUP = 4                    # strips per j-outer group (PSUM bank phasing)

_BF16 = ml_dtypes.bfloat16


def _strip_mk(s: int) -> tuple[int, int]:
    """(valid output rows, input rows) of strip s."""
    m = min(STRIP, OH - s * STRIP)
    return m, m + KH - 1


def _build_program(bias_val: float) -> bass.Bass:
    nc = bacc_mod.Bacc("TRN2", target_bir_lowering=False)

    x_d = nc.dram_tensor("xs", [128, N_PAIRS, 2 * SW], mybir.dt.bfloat16,
                         kind="ExternalInput")
    t_d = nc.dram_tensor("tmat", [128, KW * 128], mybir.dt.bfloat16,
                         kind="ExternalInput")
    y_d = nc.dram_tensor("y", [N_PAIRS, 128, 2 * CW], mybir.dt.bfloat16,
                         kind="ExternalOutput")

    with tile.TileContext(nc) as tc:
        with (
            tc.tile_pool(name="const", bufs=1) as constp,
            tc.tile_pool(name="xg", bufs=N_PAIRS) as xgp,
            tc.tile_pool(name="yg", bufs=N_PAIRS) as ygp,
            tc.tile_pool(name="ps", bufs=8, space="PSUM") as psp,
        ):
            t_sb = constp.tile([128, KW * 128], mybir.dt.bfloat16)

            xg_tiles = [
                xgp.tile([128, 2 * SW], mybir.dt.bfloat16, name="xg", tag="xg")
                for _ in range(N_PAIRS)
            ]
            nc.sync.dma_start(t_sb[:, :256], t_d[:, :256])
            nc.sync.dma_start(xg_tiles[0][:, :SW], x_d[:, 0, :SW])
            nc.sync.dma_start(xg_tiles[0][:, SW:], x_d[:, 0, SW:])
            nc.sync.dma_start(xg_tiles[1][:, :], x_d[:, 1, :])
            nc.sync.dma_start(t_sb[:, 256:], t_d[:, 256:])
            for g in range(2, N_PAIRS):
                nc.sync.dma_start(xg_tiles[g][:, :], x_d[:, g, :])

            yg_tiles = {}
            bounds = list(range(0, N_STRIPS - 6, GROUP)) + [N_STRIPS - 6]
            for gi, b0 in enumerate(bounds):
                b1 = bounds[gi + 1] if gi + 1 < len(bounds) else N_STRIPS
                strips = list(range(b0, b1))

                ps_tiles = {}
                for s in strips:
                    ps_tiles[s] = psp.tile([128, CW], mybir.dt.float32,
                                           name="ps", tag="ps")

                j_order = range(KW) if gi % 2 == 0 else range(KW - 1, -1, -1)
                for jj, j in enumerate(j_order):
                    for s in strips:
                        m, k = _strip_mk(s)
                        mw = 128 if m == STRIP else m
                        xg = xg_tiles[s // 2]
                        off = (s % 2) * SW
                        nc.tensor.matmul(
                            ps_tiles[s][:mw, :],
                            t_sb[:k, j * 128:j * 128 + mw],
                            xg[:k, off + j:off + j + CW],
                            start=(jj == 0),
                            stop=(jj == KW - 1),
                        )

                for s in strips:
                    m, _ = _strip_mk(s)
                    g, h = s // 2, s % 2
                    if h == 0:
                        yg = ygp.tile([128, 2 * CW], mybir.dt.bfloat16,
                                      name="yg", tag="yg")
                        yg_tiles[g] = yg
                    else:
                        yg = yg_tiles[g]
                    dst = yg[:m, h * CW:(h + 1) * CW]
                    src = ps_tiles[s][:m, :]
                    if s % 2 == 0:
                        nc.scalar.activation(
                            dst, src, mybir.ActivationFunctionType.Copy,
                            bias=float(bias_val),
                        )
                    else:
                        nc.vector.tensor_scalar_add(dst, src, float(bias_val))
                    if h == 1 or s == N_STRIPS - 1:
                        nc.sync.dma_start(y_d[g, :, :], yg[:, :])

    nc.compile()
    nc.finalize()
    return nc


def _toeplitz(weight: np.ndarray) -> np.ndarray:
    """[128, 7*128] bf16; block j holds T_j[k, m] = W[k-m, j] (band 0<=k-m<7)."""
    t = np.zeros((128, KW * 128), np.float32)
    for j in range(KW):
        for i in range(KH):
            mm = np.arange(0, 128 - i)
            t[mm + i, j * 128 + mm] = weight[i, j]
    return t.astype(_BF16)


def _pack_shard(x_bf: np.ndarray, c0: int) -> np.ndarray:
    """[128, 17, 2*518] bf16 (pair-inner): two strips per partition line."""
    valid = min(SW, W - c0)
    xs = np.zeros((H + 2 * STRIP, SW), _BF16)  # row padding for edge strips
    xs[:H, :valid] = x_bf[:, c0:c0 + valid]
    packed = np.zeros((N_PAIRS, 128, 2 * SW), _BF16)
    for g in range(N_PAIRS):
        packed[g, :, :SW] = xs[2 * g * STRIP: 2 * g * STRIP + 128]
        packed[g, :, SW:] = xs[(2 * g + 1) * STRIP: (2 * g + 1) * STRIP + 128]
    return np.ascontiguousarray(packed.transpose(1, 0, 2))


def _unpack_out(y_packed: np.ndarray) -> np.ndarray:
    """[17, 128, 1024] bf16 (128-row padded) -> [4090, 512] f32."""
    out = np.empty((OH, CW), np.float32)
    for s in range(N_STRIPS):
        m, _ = _strip_mk(s)
        g, h = s // 2, s % 2
        out[s * STRIP: s * STRIP + m, :] = \
            y_packed[g, :m, h * CW:(h + 1) * CW].astype(np.float32)
    return out


def kernel(x: np.ndarray, weight: np.ndarray, bias: np.ndarray) -> np.ndarray:
    x = np.asarray(x, dtype=np.float32)
    weight = np.asarray(weight, dtype=np.float32)
    bias = np.asarray(bias, dtype=np.float32)

    tmat = _toeplitz(weight)
    x_bf = x.astype(_BF16)

    in_maps = []
    for c in range(NCORES):
        in_maps.append({"xs": _pack_shard(x_bf, CW * c), "tmat": tmat})

    nc = _build_program(float(bias[0]))

    trace = bool(int(os.environ.get("CONV_KERNEL_TRACE", "0")))
    res = run_bass_kernel_spmd(nc, in_maps, core_ids=list(range(NCORES)),
                               trace=trace)
    if trace:
        kernel.last_exec_time_ns = res.exec_time_ns

    cols = []
    for c in range(NCORES):
        valid_out = min(CW, OW - CW * c)
        cols.append(_unpack_out(np.asarray(res.results[c]["y"]))[:, :valid_out])
    return np.concatenate(cols, axis=1).astype(np.float32)
